# revision 11
# baseline (speedup 1.0000x reference)
"""Trainium2 Bass kernel: CLUTRR-style GNN message passing (nn_CLUTRRV4).

Data-parallel across 8 NeuronCores (256 samples/core). Per core, samples are
packed 4-per-group (4 x 32 entity slots = 128 partitions). Entity states stay
SBUF-resident for all 8 message-passing steps; gather/scatter are expressed as
one-hot matmuls with the one-hot matrices generated on-chip from int16 index
tiles via is_equal. Matmuls run in float32r (full-rate PE mode, ~1e-4 matmul
accuracy); the state S is kept in fp32 master form with a f32r shadow copy.
N=128 matmuls are widened to N=256 via step-0 duplicated rhs APs to stay in
the f32r full-rate regime.
"""
import sys
import numpy as np

if "/opt/trn_rl_repo" not in sys.path:
    sys.path.append("/opt/trn_rl_repo")

N_ENT, N_REL, D, E = 32, 20, 128, 64
N_STEPS = 8
N_CORES = 8
P = 128
GRP = 4  # samples per group


def _patch_ldw_opt():
    import os
    if os.environ.get("BASS_LDW_OPT") != "1":
        return
    from concourse import bass_utils as bu
    if getattr(bu, "_ldw_opt_patched", False):
        return
    orig = bu.run_command

    def run_command_ldw(cmd, *a, **kw):
        if isinstance(cmd, list):
            cmd = [c.replace("--enable-ldw-opt=false", "--enable-ldw-opt=true")
                   if isinstance(c, str) else c for c in cmd]
        return orig(cmd, *a, **kw)

    bu.run_command = run_command_ldw
    bu._ldw_opt_patched = True


def _build_nc(b_core, n_steps, use_gelu=True):
    from concourse import bacc, mybir
    from concourse.tile import TileContext
    from concourse.masks import make_identity

    f32 = mybir.dt.float32
    f32r = mybir.dt.float32r
    i16 = mybir.dt.int16
    AF = mybir.ActivationFunctionType
    OP = mybir.AluOpType
    act_fn = AF.Gelu if use_gelu else AF.Identity

    G = b_core // GRP
    NPAIR = G // 2
    assert G % 4 == 0, "group count must be a multiple of 4 for rel/indeg packing"

    nc = bacc.Bacc()

    def din(name, shape, dtype=f32):
        return nc.declare_dram_parameter(name, list(shape), dtype, isOutput=False)

    d_s0 = din("s0", (P, G * P))
    d_gs = din("gsrc", (G, P, 256), i16)
    d_gt = din("gtgt", (G, P, 256), i16)
    d_gtc = din("gtc", (P, 2 * G), i16)
    d_rel = din("reloh", (G // 4, P, 256), f32r)
    d_rt4 = din("reltab4", (P, 256), f32r)
    d_ind = din("indeg", (G // 4, P, P), f32r)
    d_b2r = din("b2row", (P, P), f32r)
    d_qoh = din("qoh", (G, P, 8))
    d_w1ac = din("w1ac", (P, 512), f32r)
    d_w2m = din("w2m", (P, 256), f32r)
    d_w1u = din("w1u", (P, 512), f32r)
    d_w2u = din("w2u", (P, 256), f32r)
    d_b1u = din("b1u", (P, 2))
    d_b2u = din("b2u", (P, 1))
    d_cw1 = din("cw1", (P, 256))
    d_cb1 = din("cb1", (P, 1))
    d_cw2 = din("cw2", (P, 20))
    d_cb2 = din("cb2", (20, 1))
    d_out = nc.declare_dram_parameter("out", [20, b_core], f32, isOutput=True)

    with TileContext(nc) as tc:
        with (
            tc.tile_pool(name="c", bufs=1) as cp,
            tc.tile_pool(name="w", bufs=3) as wp,
            tc.tile_pool(name="pA", bufs=2, space="PSUM") as pA,
            tc.tile_pool(name="pH1", bufs=2, space="PSUM") as pH1,
            tc.tile_pool(name="pM", bufs=2, space="PSUM") as pM,
            tc.tile_pool(name="pG", bufs=1, space="PSUM") as pG,
            tc.tile_pool(name="pS", bufs=1, space="PSUM") as pS,
        ):
            def cload(name, shape, dram, dtype=f32):
                t = cp.tile(list(shape), dtype, tag=name)
                nc.sync.dma_start(t[:], dram[:])
                return t

            w1ac = cload("w1ac", (P, 512), d_w1ac, f32r)
            w2m = cload("w2m", (P, 256), d_w2m, f32r)
            w1u = cload("w1u", (P, 512), d_w1u, f32r)
            w2u = cload("w2u", (P, 256), d_w2u, f32r)
            rt4 = cload("rt4", (P, 256), d_rt4, f32r)
            b2r = cload("b2r", (P, P), d_b2r, f32r)
            b1u = cload("b1u", (P, 2), d_b1u)
            b2u = cload("b2u", (P, 1), d_b2u)
            cw1 = cload("cw1", (P, 256), d_cw1)
            cb1 = cload("cb1", (P, 1), d_cb1)
            cw2 = cload("cw2", (P, 20), d_cw2)
            cb2 = cload("cb2", (20, 1), d_cb2)
            gtc = cload("gtc", (P, 2 * G), d_gtc, i16)

            eiota = cp.tile([P, 1], i16, tag="eiota")
            nc.gpsimd.iota(eiota[:], pattern=[[0, 1]], base=0, channel_multiplier=1)
            fiota = cp.tile([P, P], i16, tag="fiota")
            nc.gpsimd.iota(fiota[:], pattern=[[1, P]], base=0, channel_multiplier=0)
            ident = cp.tile([P, P], f32, tag="ident")
            make_identity(nc, ident[:])
            outsb = cp.tile([20, b_core], f32, tag="outsb")

            S, SR = [], []
            GS, GT, QOH = [None] * G, [None] * G, [None] * G
            RELP, INDP = [None] * (G // 4), [None] * (G // 4)
            for p in range(NPAIR):
                if p % 2 == 0:
                    j = p // 2
                    t = cp.tile([P, 256], f32r, tag=f"rp{j}")
                    nc.sync.dma_start(t[:], d_rel[j])
                    RELP[j] = t
                    t = cp.tile([P, P], f32r, tag=f"ip{j}")
                    nc.sync.dma_start(t[:], d_ind[j])
                    INDP[j] = t
                t = cp.tile([P, 256], f32, tag=f"S{p}")
                nc.sync.dma_start(t[:], d_s0[:, p * 256:(p + 1) * 256])
                S.append(t)
                t2 = cp.tile([P, 256], f32r, tag=f"Sr{p}")
                nc.vector.tensor_copy(t2[:], t[:])
                SR.append(t2)
                for g in (2 * p, 2 * p + 1):
                    t = cp.tile([P, 256], i16, tag=f"gs{g}")
                    nc.sync.dma_start(t[:], d_gs[g])
                    GS[g] = t
                    t = cp.tile([P, 256], i16, tag=f"gt{g}")
                    nc.sync.dma_start(t[:], d_gt[g])
                    GT[g] = t
                    t = cp.tile([P, 8], f32, tag=f"q{g}")
                    nc.sync.dma_start(t[:], d_qoh[g])
                    QOH[g] = t

            mm = nc.tensor.matmul

            def dup2(ap_):
                """(K, n) AP -> (K, 2, n) with step-0 middle dim (rhs widening)."""
                k, n = ap_.shape
                return ap_[:, None, :].to_broadcast([k, 2, n])

            for t_step in range(n_steps):
                for p in range(NPAIR):
                    agg = pG.tile([P, 512], f32, tag="agg")
                    for gi in range(2):
                        g = 2 * p + gi
                        rb = (g % 4) * 32
                        # A = [S@W1a | S@W1c] in natural (slot-major) layout
                        aps = pA.tile([P, 512], f32, tag="aps")
                        mm(aps[:], lhsT=SR[p][:, gi * P:(gi + 1) * P], rhs=w1ac[:],
                           start=True, stop=True)
                        asb = wp.tile([P, 512], f32r, tag="asb")
                        nc.vector.tensor_copy(asb[:], aps[:])
                        # ent-major one-hots (DVE)
                        ohs = wp.tile([P, 256], f32r, tag="ohs")
                        nc.vector.tensor_tensor(
                            ohs[:], GS[g][:], eiota[:].to_broadcast([P, 256]),
                            op=OP.is_equal)
                        oht = wp.tile([P, 256], f32r, tag="oht")
                        nc.vector.tensor_tensor(
                            oht[:], GT[g][:], eiota[:].to_broadcast([P, 256]),
                            op=OP.is_equal)
                        # h1 = rel_bias + onehot_src@A + onehot_tgt@Bt (per featchunk)
                        h1 = pH1.tile([P, 512], f32, tag="h1")
                        for F in range(2):
                            o = h1[:, F * 256:(F + 1) * 256]
                            mm(o, lhsT=rt4[rb:rb + 20, F * P:(F + 1) * P],
                               rhs=RELP[g // 4][rb:rb + 20, :], start=True, stop=False,
                               tile_position=(rb, 0))
                            mm(o, lhsT=asb[:, F * P:(F + 1) * P], rhs=ohs[:],
                               start=False, stop=False)
                            mm(o, lhsT=asb[:, 256 + F * P:256 + (F + 1) * P],
                               rhs=oht[:], start=False, stop=True)
                        h1g = wp.tile([P, 512], f32r, tag="h1g")
                        nc.scalar.activation(h1g[:], h1[:], act_fn)
                        # msg layer 2, emitted edge-major; rhs duplicated to N=256
                        msg = pM.tile([P, 512], f32, tag="msg")
                        for ec in range(2):
                            o = msg[:, ec * 256:(ec + 1) * 256]
                            for F in range(2):
                                mm(o, lhsT=h1g[:, F * 256 + ec * P:F * 256 + (ec + 1) * P],
                                   rhs=dup2(w2m[:, F * P:(F + 1) * P]),
                                   start=(F == 0), stop=(F == 1))
                        msb = wp.tile([P, 256], f32r, tag="msb")
                        mv = msg[:].rearrange("p (e t q) -> p e t q", t=2, q=P)
                        nc.vector.tensor_copy(msb[:], mv[:, :, 0, :])
                        # edge-major masked tgt one-hot (DVE)
                        ohe = wp.tile([P, 256], f32r, tag="ohe")
                        for ec in range(2):
                            col = gtc[:, g * 2 + ec:g * 2 + ec + 1]
                            nc.vector.tensor_tensor(
                                ohe[:, ec * P:(ec + 1) * P],
                                col.to_broadcast([P, P]), fiota[:], op=OP.is_equal)
                        # scatter-add + msg_b2*indegree fold (rhs duplicated)
                        o = agg[:, gi * 256:(gi + 1) * 256]
                        mm(o, lhsT=msb[:, 0:P], rhs=dup2(ohe[:, 0:P]),
                           start=True, stop=False)
                        mm(o, lhsT=msb[:, P:256], rhs=dup2(ohe[:, P:256]),
                           start=False, stop=False)
                        mm(o, lhsT=b2r[rb:rb + 1, :],
                           rhs=dup2(INDP[g // 4][rb:rb + 1, :]),
                           start=False, stop=True, tile_position=(rb, 0))
                    # update MLP over the pair (256 slot cols)
                    gsb = wp.tile([P, 256], f32r, tag="gsb")
                    gv = agg[:].rearrange("p (g t q) -> p g t q", t=2, q=P)
                    nc.scalar.copy(gsb[:], gv[:, :, 0, :])
                    h3 = pH1.tile([P, 512], f32, tag="h1")
                    for mc in range(2):
                        o = h3[:, mc * 256:(mc + 1) * 256]
                        mm(o, lhsT=w1u[:, mc * P:(mc + 1) * P], rhs=SR[p][:],
                           start=True, stop=False)
                        mm(o, lhsT=w1u[:, 256 + mc * P:256 + (mc + 1) * P], rhs=gsb[:],
                           start=False, stop=True)
                    h3g = wp.tile([P, 512], f32r, tag="h3g")
                    for mc in range(2):
                        nc.scalar.activation(
                            h3g[:, mc * 256:(mc + 1) * 256],
                            h3[:, mc * 256:(mc + 1) * 256], act_fn,
                            bias=b1u[:, mc:mc + 1])
                    sn = pS.tile([P, 256], f32, tag="sn")
                    for kc in range(2):
                        mm(sn[:], lhsT=w2u[:, kc * P:(kc + 1) * P],
                           rhs=h3g[:, kc * 256:(kc + 1) * 256],
                           start=(kc == 0), stop=(kc == 1))
                    # S += sn + b2u (fp32 master), then refresh the f32r shadow
                    nc.vector.scalar_tensor_tensor(
                        out=S[p][:], in0=sn[:], scalar=b2u[:, 0:1], in1=S[p][:],
                        op0=OP.add, op1=OP.add)
                    nc.vector.tensor_copy(SR[p][:], S[p][:])

            # classifier head (fp32 throughout; tiny)
            nbatch = (G + 15) // 16
            for bq in range(nbatch):
                jn = min(16, G - bq * 16)
                qps = pH1.tile([P, P], f32, tag="h1")
                for j in range(jn):
                    g = bq * 16 + j
                    p2, gi = divmod(g, 2)
                    stp = pA.tile([P, P], f32, tag="aps")
                    nc.tensor.transpose(stp[:], S[p2][:, gi * P:(gi + 1) * P], ident[:])
                    sts = wp.tile([P, P], f32, tag="sts")
                    nc.vector.tensor_copy(sts[:], stp[:])
                    mm(qps[:, j * 8:(j + 1) * 8], lhsT=sts[:], rhs=QOH[g][:],
                       start=True, stop=True)
                qcat = wp.tile([P, P], f32, tag="qcat")
                nc.vector.tensor_copy(qcat[:, 0:jn * 8], qps[:, 0:jn * 8])
                qv = qcat[:, 0:jn * 8].rearrange("p (g t f) -> p g t f", t=2, f=4)
                ncols = jn * 4
                hps = pM.tile([P, ncols], f32, tag="msg")
                mm(hps[:], lhsT=cw1[:, 0:P], rhs=qv[:, :, 0, :], start=True, stop=False)
                mm(hps[:], lhsT=cw1[:, P:256], rhs=qv[:, :, 1, :], start=False, stop=True)
                hg = wp.tile([P, ncols], f32, tag="hg")
                nc.scalar.activation(hg[:], hps[:], act_fn, bias=cb1[:, 0:1])
                ops_ = pG.tile([20, ncols], f32, tag="agg")
                mm(ops_[:], lhsT=cw2[:], rhs=hg[:], start=True, stop=True)
                nc.scalar.activation(
                    outsb[:, bq * 64:bq * 64 + ncols], ops_[:], AF.Identity,
                    bias=cb2[:, 0:1])
            nc.sync.dma_start(d_out[:], outsb[:])

    nc.finalize()
    return nc


def _host_prep_shared(inp, b_core):
    f = np.float32
    ee = np.asarray(inp["entity_embed"], f)
    w1 = np.asarray(inp["msg_W1"], f)
    reltab = np.asarray(inp["rel_embed"], f) @ w1[128:256] + np.asarray(inp["msg_b1"], f)
    rt4 = np.zeros((P, 256), f)
    b2r = np.zeros((P, P), f)
    for b in range(4):
        rt4[b * 32:b * 32 + 20] = reltab
        b2r[b * 32] = np.asarray(inp["msg_b2"], f)
    w2m_ = np.asarray(inp["msg_W2"], f)
    w1u_ = np.asarray(inp["upd_W1"], f)
    w2u_ = np.asarray(inp["upd_W2"], f)
    cw1_ = np.asarray(inp["cls_W1"], f)
    return {
        "s0": np.tile(ee.T, (1, b_core)).astype(f),
        "reltab4": rt4,
        "b2row": b2r,
        "w1ac": np.concatenate([w1[0:128], w1[256:384]], axis=1).astype(f),
        "w2m": np.concatenate([w2m_[0:128], w2m_[128:256]], axis=1).astype(f),
        "w1u": np.concatenate(
            [w1u_[0:128, 0:128], w1u_[0:128, 128:256],
             w1u_[128:256, 0:128], w1u_[128:256, 128:256]], axis=1).astype(f),
        "w2u": np.concatenate([w2u_[0:128], w2u_[128:256]], axis=1).astype(f),
        "b1u": np.asarray(inp["upd_b1"], f).reshape(2, 128).T.copy(),
        "b2u": np.asarray(inp["upd_b2"], f).reshape(128, 1).copy(),
        "cw1": np.concatenate([cw1_[0:128], cw1_[128:256]], axis=1).astype(f),
        "cb1": np.asarray(inp["cls_b1"], f).reshape(128, 1).copy(),
        "cw2": np.asarray(inp["cls_W2"], f).copy(),
        "cb2": np.asarray(inp["cls_b2"], f).reshape(20, 1).copy(),
    }


def _host_prep_core(inp, c, b_core):
    f = np.float32
    sl = slice(c * b_core, (c + 1) * b_core)
    src = np.asarray(inp["edge_src"])[sl].astype(np.int64)
    tgt = np.asarray(inp["edge_tgt"])[sl].astype(np.int64)
    rel = np.asarray(inp["edge_rel"])[sl].astype(np.int64)
    ne = np.asarray(inp["n_edges"])[sl].astype(np.int64)
    qs = np.asarray(inp["query_src"])[sl].astype(np.int64)
    qt = np.asarray(inp["query_tgt"])[sl].astype(np.int64)
    G = b_core // GRP

    mask = (np.arange(E)[None, :] < ne[:, None])
    soff = (np.arange(b_core) % GRP)[:, None] * 32
    gs = np.where(mask, soff + src, 255).astype(np.int16).reshape(G, 256)
    gt = np.where(mask, soff + tgt, 255).astype(np.int16).reshape(G, 256)

    relg = rel.reshape(G, 256)
    reloh = np.zeros((G // 4, P, 256), f)
    oh = (relg[:, None, :] == np.arange(20)[None, :, None]).astype(f)
    reloh.reshape(G // 4, 4, 32, 256)[:, :, :20] = oh.reshape(G // 4, 4, 20, 256)

    ind = np.zeros((b_core, 32), f)
    np.add.at(ind, (np.repeat(np.arange(b_core), E), tgt.ravel()),
              mask.ravel().astype(f))
    indp = np.zeros((G // 4, P, P), f)
    indp.reshape(G // 4, 4, 32, P)[:, :, 0, :] = ind.reshape(G // 4, 4, P)

    qoh = np.zeros((G, P, 8), f)
    s_all = np.arange(b_core)
    gidx = s_all // GRP
    sg = s_all % GRP
    qoh[gidx, sg * 32 + qs, sg] = 1.0
    qoh[gidx, sg * 32 + qt, 4 + sg] = 1.0

    return {
        "gsrc": np.ascontiguousarray(np.broadcast_to(gs[:, None, :], (G, P, 256))),
        "gtgt": np.ascontiguousarray(np.broadcast_to(gt[:, None, :], (G, P, 256))),
        "gtc": np.ascontiguousarray(gt.reshape(2 * G, P).T),
        "reloh": reloh,
        "indeg": indp,
        "qoh": qoh,
    }


_CACHE = {}


def kernel(**inputs):
    b = np.asarray(inputs["edge_src"]).shape[0]
    b_core = b // N_CORES
    _patch_ldw_opt()
    key = b_core
    if key not in _CACHE:
        _CACHE[key] = _build_nc(b_core, N_STEPS, use_gelu=True)
    nc = _CACHE[key]

    shared = _host_prep_shared(inputs, b_core)
    in_maps = []
    for c in range(N_CORES):
        m = dict(shared)
        m.update(_host_prep_core(inputs, c, b_core))
        in_maps.append(m)

    from concourse.bass_utils import run_bass_kernel_spmd
    res = run_bass_kernel_spmd(nc, in_maps, core_ids=list(range(N_CORES)))
    out = np.concatenate([r["out"].T for r in res.results], axis=0)
    return np.ascontiguousarray(out, dtype=np.float32)


# revision 13
# speedup vs baseline: 1.6353x; 1.6353x over previous
"""Trainium2 Bass kernel: CLUTRR-style GNN message passing (nn_CLUTRRV4).

Data-parallel across 8 NeuronCores (256 samples/core). Per core, samples are
packed 4-per-group (4 x 32 entity slots = 128 partitions). Entity states stay
SBUF-resident for all 8 message-passing steps; gather/scatter are expressed as
one-hot matmuls with the one-hot matrices generated on-chip from int16 index
tiles via is_equal. Matmuls run in float16 (full-rate PE + fast weight load, ~5e-4 matmul
accuracy); the state S is kept in fp32 master form with an fp16 shadow copy.
N=128 matmuls are widened to N=256 via step-0 duplicated rhs APs to stay in
the f32r full-rate regime.
"""
import sys
import numpy as np

if "/opt/trn_rl_repo" not in sys.path:
    sys.path.append("/opt/trn_rl_repo")

N_ENT, N_REL, D, E = 32, 20, 128, 64
N_STEPS = 8
N_CORES = 8
P = 128
GRP = 4  # samples per group


def _patch_ldw_opt():
    import os
    if os.environ.get("BASS_LDW_OPT") != "1":
        return
    from concourse import bass_utils as bu
    if getattr(bu, "_ldw_opt_patched", False):
        return
    orig = bu.run_command

    def run_command_ldw(cmd, *a, **kw):
        if isinstance(cmd, list):
            cmd = [c.replace("--enable-ldw-opt=false", "--enable-ldw-opt=true")
                   if isinstance(c, str) else c for c in cmd]
        return orig(cmd, *a, **kw)

    bu.run_command = run_command_ldw
    bu._ldw_opt_patched = True


def _build_nc(b_core, n_steps, use_gelu=True):
    from concourse import bacc, mybir
    from concourse.tile import TileContext
    from concourse.masks import make_identity

    f32 = mybir.dt.float32
    f32r = mybir.dt.float16
    i16 = mybir.dt.int16
    AF = mybir.ActivationFunctionType
    OP = mybir.AluOpType
    act_fn = AF.Gelu if use_gelu else AF.Identity

    G = b_core // GRP
    NPAIR = G // 2
    assert G % 4 == 0, "group count must be a multiple of 4 for rel/indeg packing"

    nc = bacc.Bacc()

    def din(name, shape, dtype=f32):
        return nc.declare_dram_parameter(name, list(shape), dtype, isOutput=False)

    d_s0 = din("s0", (P, G * P))
    d_gs = din("gsrc", (G, P, 256), i16)
    d_gt = din("gtgt", (G, P, 256), i16)
    d_gtc = din("gtc", (P, 2 * G), i16)
    d_rel = din("reloh", (G // 4, P, 256), f32r)
    d_rt4 = din("reltab4", (P, 256), f32r)
    d_ind = din("indeg", (G // 4, P, P), f32r)
    d_b2r = din("b2row", (P, P), f32r)
    d_qoh = din("qoh", (G, P, 8))
    d_w1ac = din("w1ac", (P, 512), f32r)
    d_w2m = din("w2m", (P, 256), f32r)
    d_w1u = din("w1u", (P, 512), f32r)
    d_w2u = din("w2u", (P, 256), f32r)
    d_b1u = din("b1u", (P, 2))
    d_b2u = din("b2u", (P, 1))
    d_cw1 = din("cw1", (P, 256))
    d_cb1 = din("cb1", (P, 1))
    d_cw2 = din("cw2", (P, 20))
    d_cb2 = din("cb2", (20, 1))
    d_out = nc.declare_dram_parameter("out", [20, b_core], f32, isOutput=True)

    with TileContext(nc) as tc:
        with (
            tc.tile_pool(name="c", bufs=1) as cp,
            tc.tile_pool(name="w", bufs=3) as wp,
            tc.tile_pool(name="pA", bufs=2, space="PSUM") as pA,
            tc.tile_pool(name="pH1", bufs=2, space="PSUM") as pH1,
            tc.tile_pool(name="pM", bufs=1, space="PSUM") as pM,
            tc.tile_pool(name="pG", bufs=1, space="PSUM") as pG,
            tc.tile_pool(name="pH3", bufs=1, space="PSUM") as pH3,
            tc.tile_pool(name="pS", bufs=1, space="PSUM") as pS,
        ):
            def cload(name, shape, dram, dtype=f32):
                t = cp.tile(list(shape), dtype, tag=name)
                nc.sync.dma_start(t[:], dram[:])
                return t

            w1ac = cload("w1ac", (P, 512), d_w1ac, f32r)
            w2m = cload("w2m", (P, 256), d_w2m, f32r)
            w1u = cload("w1u", (P, 512), d_w1u, f32r)
            w2u = cload("w2u", (P, 256), d_w2u, f32r)
            rt4 = cload("rt4", (P, 256), d_rt4, f32r)
            b2r = cload("b2r", (P, P), d_b2r, f32r)
            b1u = cload("b1u", (P, 2), d_b1u)
            b2u = cload("b2u", (P, 1), d_b2u)
            cw1 = cload("cw1", (P, 256), d_cw1)
            cb1 = cload("cb1", (P, 1), d_cb1)
            cw2 = cload("cw2", (P, 20), d_cw2)
            cb2 = cload("cb2", (20, 1), d_cb2)
            gtc = cload("gtc", (P, 2 * G), d_gtc, i16)

            eiota = cp.tile([P, 1], i16, tag="eiota")
            nc.gpsimd.iota(eiota[:], pattern=[[0, 1]], base=0, channel_multiplier=1)
            fiota = cp.tile([P, P], i16, tag="fiota")
            nc.gpsimd.iota(fiota[:], pattern=[[1, P]], base=0, channel_multiplier=0)
            ident = cp.tile([P, P], f32, tag="ident")
            make_identity(nc, ident[:])
            outsb = cp.tile([20, b_core], f32, tag="outsb")

            S, SR = [], []
            GS, GT, QOH = [None] * G, [None] * G, [None] * G
            RELP, INDP = [None] * (G // 4), [None] * (G // 4)
            for p in range(NPAIR):
                if p % 2 == 0:
                    j = p // 2
                    t = cp.tile([P, 256], f32r, tag=f"rp{j}")
                    nc.sync.dma_start(t[:], d_rel[j])
                    RELP[j] = t
                    t = cp.tile([P, P], f32r, tag=f"ip{j}")
                    nc.sync.dma_start(t[:], d_ind[j])
                    INDP[j] = t
                t = cp.tile([P, 256], f32, tag=f"S{p}")
                nc.sync.dma_start(t[:], d_s0[:, p * 256:(p + 1) * 256])
                S.append(t)
                t2 = cp.tile([P, 256], f32r, tag=f"Sr{p}")
                nc.vector.tensor_copy(t2[:], t[:])
                SR.append(t2)
                for g in (2 * p, 2 * p + 1):
                    t = cp.tile([P, 256], i16, tag=f"gs{g}")
                    nc.sync.dma_start(t[:], d_gs[g])
                    GS[g] = t
                    t = cp.tile([P, 256], i16, tag=f"gt{g}")
                    nc.sync.dma_start(t[:], d_gt[g])
                    GT[g] = t
                    t = cp.tile([P, 8], f32, tag=f"q{g}")
                    nc.sync.dma_start(t[:], d_qoh[g])
                    QOH[g] = t

            mm = nc.tensor.matmul

            def dup2(ap_):
                """(K, n) AP -> (K, 2, n) with step-0 middle dim (rhs widening)."""
                k, n = ap_.shape
                return ap_[:, None, :].to_broadcast([k, 2, n])

            for t_step in range(n_steps):
                for p in range(NPAIR):
                    agg = pG.tile([P, 512], f32, tag="agg")
                    for gi in range(2):
                        g = 2 * p + gi
                        rb = (g % 4) * 32
                        # A = [S@W1a | S@W1c] in natural (slot-major) layout
                        aps = pA.tile([P, 512], f32, tag="aps")
                        mm(aps[:], lhsT=SR[p][:, gi * P:(gi + 1) * P], rhs=w1ac[:],
                           start=True, stop=True)
                        asb = wp.tile([P, 512], f32r, tag="asb")
                        nc.vector.tensor_copy(asb[:], aps[:])
                        # ent-major one-hots (DVE)
                        ohs = wp.tile([P, 256], f32r, tag="ohs")
                        nc.vector.tensor_tensor(
                            ohs[:], GS[g][:], eiota[:].to_broadcast([P, 256]),
                            op=OP.is_equal)
                        oht = wp.tile([P, 256], f32r, tag="oht")
                        nc.vector.tensor_tensor(
                            oht[:], GT[g][:], eiota[:].to_broadcast([P, 256]),
                            op=OP.is_equal)
                        # h1 = rel_bias + onehot_src@A + onehot_tgt@Bt (per featchunk)
                        h1 = pH1.tile([P, 512], f32, tag="h1")
                        for F in range(2):
                            o = h1[:, F * 256:(F + 1) * 256]
                            mm(o, lhsT=rt4[rb:rb + 20, F * P:(F + 1) * P],
                               rhs=RELP[g // 4][rb:rb + 20, :], start=True, stop=False,
                               tile_position=(rb, 0))
                            mm(o, lhsT=asb[:, F * P:(F + 1) * P], rhs=ohs[:],
                               start=False, stop=False)
                            mm(o, lhsT=asb[:, 256 + F * P:256 + (F + 1) * P],
                               rhs=oht[:], start=False, stop=True)
                        h1g = wp.tile([P, 512], f32r, tag="h1g")
                        nc.scalar.activation(h1g[:], h1[:], act_fn)
                        # msg layer 2, emitted edge-major; rhs duplicated to N=256
                        msg = pM.tile([P, 512], f32, tag="msg")
                        for ec in range(2):
                            o = msg[:, ec * 256:(ec + 1) * 256]
                            for F in range(2):
                                mm(o, lhsT=h1g[:, F * 256 + ec * P:F * 256 + (ec + 1) * P],
                                   rhs=dup2(w2m[:, F * P:(F + 1) * P]),
                                   start=(F == 0), stop=(F == 1))
                        msb = wp.tile([P, 256], f32r, tag="msb")
                        mv = msg[:].rearrange("p (e t q) -> p e t q", t=2, q=P)
                        nc.vector.tensor_copy(msb[:], mv[:, :, 0, :])
                        # edge-major masked tgt one-hot (DVE)
                        ohe = wp.tile([P, 256], f32r, tag="ohe")
                        for ec in range(2):
                            col = gtc[:, g * 2 + ec:g * 2 + ec + 1]
                            nc.vector.tensor_tensor(
                                ohe[:, ec * P:(ec + 1) * P],
                                col.to_broadcast([P, P]), fiota[:], op=OP.is_equal)
                        # scatter-add + msg_b2*indegree fold (rhs duplicated)
                        o = agg[:, gi * 256:(gi + 1) * 256]
                        mm(o, lhsT=msb[:, 0:P], rhs=dup2(ohe[:, 0:P]),
                           start=True, stop=False)
                        mm(o, lhsT=msb[:, P:256], rhs=dup2(ohe[:, P:256]),
                           start=False, stop=False)
                        mm(o, lhsT=b2r[rb:rb + 1, :],
                           rhs=dup2(INDP[g // 4][rb:rb + 1, :]),
                           start=False, stop=True, tile_position=(rb, 0))
                    # update MLP over the pair (256 slot cols)
                    gsb = wp.tile([P, 256], f32r, tag="gsb")
                    gv = agg[:].rearrange("p (g t q) -> p g t q", t=2, q=P)
                    nc.scalar.copy(gsb[:], gv[:, :, 0, :])
                    h3 = pH3.tile([P, 512], f32, tag="h3")
                    for mc in range(2):
                        o = h3[:, mc * 256:(mc + 1) * 256]
                        mm(o, lhsT=w1u[:, mc * P:(mc + 1) * P], rhs=SR[p][:],
                           start=True, stop=False)
                        mm(o, lhsT=w1u[:, 256 + mc * P:256 + (mc + 1) * P], rhs=gsb[:],
                           start=False, stop=True)
                    h3g = wp.tile([P, 512], f32r, tag="h3g")
                    for mc in range(2):
                        nc.scalar.activation(
                            h3g[:, mc * 256:(mc + 1) * 256],
                            h3[:, mc * 256:(mc + 1) * 256], act_fn,
                            bias=b1u[:, mc:mc + 1])
                    sn = pS.tile([P, 256], f32, tag="sn")
                    for kc in range(2):
                        mm(sn[:], lhsT=w2u[:, kc * P:(kc + 1) * P],
                           rhs=h3g[:, kc * 256:(kc + 1) * 256],
                           start=(kc == 0), stop=(kc == 1))
                    # S += sn + b2u (fp32 master), then refresh the f32r shadow
                    nc.vector.scalar_tensor_tensor(
                        out=S[p][:], in0=sn[:], scalar=b2u[:, 0:1], in1=S[p][:],
                        op0=OP.add, op1=OP.add)
                    nc.vector.tensor_copy(SR[p][:], S[p][:])

            # classifier head (fp32 throughout; tiny)
            nbatch = (G + 15) // 16
            for bq in range(nbatch):
                jn = min(16, G - bq * 16)
                qps = pH1.tile([P, P], f32, tag="h1")
                for j in range(jn):
                    g = bq * 16 + j
                    p2, gi = divmod(g, 2)
                    stp = pA.tile([P, P], f32, tag="aps")
                    nc.tensor.transpose(stp[:], S[p2][:, gi * P:(gi + 1) * P], ident[:])
                    sts = wp.tile([P, P], f32, tag="sts")
                    nc.vector.tensor_copy(sts[:], stp[:])
                    mm(qps[:, j * 8:(j + 1) * 8], lhsT=sts[:], rhs=QOH[g][:],
                       start=True, stop=True)
                qcat = wp.tile([P, P], f32, tag="qcat")
                nc.vector.tensor_copy(qcat[:, 0:jn * 8], qps[:, 0:jn * 8])
                qv = qcat[:, 0:jn * 8].rearrange("p (g t f) -> p g t f", t=2, f=4)
                ncols = jn * 4
                hps = pM.tile([P, ncols], f32, tag="msg")
                mm(hps[:], lhsT=cw1[:, 0:P], rhs=qv[:, :, 0, :], start=True, stop=False)
                mm(hps[:], lhsT=cw1[:, P:256], rhs=qv[:, :, 1, :], start=False, stop=True)
                hg = wp.tile([P, ncols], f32, tag="hg")
                nc.scalar.activation(hg[:], hps[:], act_fn, bias=cb1[:, 0:1])
                ops_ = pG.tile([20, ncols], f32, tag="agg")
                mm(ops_[:], lhsT=cw2[:], rhs=hg[:], start=True, stop=True)
                nc.scalar.activation(
                    outsb[:, bq * 64:bq * 64 + ncols], ops_[:], AF.Identity,
                    bias=cb2[:, 0:1])
            nc.sync.dma_start(d_out[:], outsb[:])

    nc.finalize()
    return nc


def _host_prep_shared(inp, b_core):
    f = np.float32
    ee = np.asarray(inp["entity_embed"], f)
    w1 = np.asarray(inp["msg_W1"], f)
    reltab = np.asarray(inp["rel_embed"], f) @ w1[128:256] + np.asarray(inp["msg_b1"], f)
    rt4 = np.zeros((P, 256), f)
    b2r = np.zeros((P, P), f)
    for b in range(4):
        rt4[b * 32:b * 32 + 20] = reltab
        b2r[b * 32] = np.asarray(inp["msg_b2"], f)
    w2m_ = np.asarray(inp["msg_W2"], f)
    w1u_ = np.asarray(inp["upd_W1"], f)
    w2u_ = np.asarray(inp["upd_W2"], f)
    cw1_ = np.asarray(inp["cls_W1"], f)
    h = np.float16
    return {
        "s0": np.tile(ee.T, (1, b_core)).astype(f),
        "reltab4": rt4.astype(h),
        "b2row": b2r.astype(h),
        "w1ac": np.concatenate([w1[0:128], w1[256:384]], axis=1).astype(h),
        "w2m": np.concatenate([w2m_[0:128], w2m_[128:256]], axis=1).astype(h),
        "w1u": np.concatenate(
            [w1u_[0:128, 0:128], w1u_[0:128, 128:256],
             w1u_[128:256, 0:128], w1u_[128:256, 128:256]], axis=1).astype(h),
        "w2u": np.concatenate([w2u_[0:128], w2u_[128:256]], axis=1).astype(h),
        "b1u": np.asarray(inp["upd_b1"], f).reshape(2, 128).T.copy(),
        "b2u": np.asarray(inp["upd_b2"], f).reshape(128, 1).copy(),
        "cw1": np.concatenate([cw1_[0:128], cw1_[128:256]], axis=1).astype(f),
        "cb1": np.asarray(inp["cls_b1"], f).reshape(128, 1).copy(),
        "cw2": np.asarray(inp["cls_W2"], f).copy(),
        "cb2": np.asarray(inp["cls_b2"], f).reshape(20, 1).copy(),
    }


def _host_prep_core(inp, c, b_core):
    f = np.float32
    sl = slice(c * b_core, (c + 1) * b_core)
    src = np.asarray(inp["edge_src"])[sl].astype(np.int64)
    tgt = np.asarray(inp["edge_tgt"])[sl].astype(np.int64)
    rel = np.asarray(inp["edge_rel"])[sl].astype(np.int64)
    ne = np.asarray(inp["n_edges"])[sl].astype(np.int64)
    qs = np.asarray(inp["query_src"])[sl].astype(np.int64)
    qt = np.asarray(inp["query_tgt"])[sl].astype(np.int64)
    G = b_core // GRP

    mask = (np.arange(E)[None, :] < ne[:, None])
    soff = (np.arange(b_core) % GRP)[:, None] * 32
    gs = np.where(mask, soff + src, 255).astype(np.int16).reshape(G, 256)
    gt = np.where(mask, soff + tgt, 255).astype(np.int16).reshape(G, 256)

    relg = rel.reshape(G, 256)
    reloh = np.zeros((G // 4, P, 256), f)
    oh = (relg[:, None, :] == np.arange(20)[None, :, None]).astype(f)
    reloh.reshape(G // 4, 4, 32, 256)[:, :, :20] = oh.reshape(G // 4, 4, 20, 256)

    ind = np.zeros((b_core, 32), f)
    np.add.at(ind, (np.repeat(np.arange(b_core), E), tgt.ravel()),
              mask.ravel().astype(f))
    indp = np.zeros((G // 4, P, P), f)
    indp.reshape(G // 4, 4, 32, P)[:, :, 0, :] = ind.reshape(G // 4, 4, P)

    qoh = np.zeros((G, P, 8), f)
    s_all = np.arange(b_core)
    gidx = s_all // GRP
    sg = s_all % GRP
    qoh[gidx, sg * 32 + qs, sg] = 1.0
    qoh[gidx, sg * 32 + qt, 4 + sg] = 1.0

    return {
        "gsrc": np.ascontiguousarray(np.broadcast_to(gs[:, None, :], (G, P, 256))),
        "gtgt": np.ascontiguousarray(np.broadcast_to(gt[:, None, :], (G, P, 256))),
        "gtc": np.ascontiguousarray(gt.reshape(2 * G, P).T),
        "reloh": reloh.astype(np.float16),
        "indeg": indp.astype(np.float16),
        "qoh": qoh,
    }


_CACHE = {}


def kernel(**inputs):
    b = np.asarray(inputs["edge_src"]).shape[0]
    b_core = b // N_CORES
    _patch_ldw_opt()
    key = b_core
    if key not in _CACHE:
        _CACHE[key] = _build_nc(b_core, N_STEPS, use_gelu=True)
    nc = _CACHE[key]

    shared = _host_prep_shared(inputs, b_core)
    in_maps = []
    for c in range(N_CORES):
        m = dict(shared)
        m.update(_host_prep_core(inputs, c, b_core))
        in_maps.append(m)

    from concourse.bass_utils import run_bass_kernel_spmd
    res = run_bass_kernel_spmd(nc, in_maps, core_ids=list(range(N_CORES)))
    out = np.concatenate([r["out"].T for r in res.results], axis=0)
    return np.ascontiguousarray(out, dtype=np.float32)
